# revision 25
# baseline (speedup 1.0000x reference)
"""Trainium2 Bass kernel for nn_DiscreteTimeNeuralGraph.

Strategy (8 NeuronCores, batch-parallel, 4 samples/core):
  - Downsample path on PE (f32r); BatchNorm batch stats via per-core partial
    sums + AllGather (cheaper than AllReduce in this stack) + local sum.
  - Main loop in fp16: X maps padded to 34x34 so all 9 depthwise taps are
    full-range.  Depthwise conv split across engines:
      groups 0,1 -> PE as 9 rect diagonal matmuls/half (psum), Pool copies
                    psum -> fp16 D tiles;
      groups 2,3 -> DVE: 9 per-channel-scalar products (fp16, 4x DVE mode)
                    into plane stacks, then 2 accumulate-DMAs (Pool SWDGE)
                    + 3 DVE adds to reduce the 9 planes.
    Channel mix (pruned 512x512, dense fp16) as K-blocked matmuls on PE
    accumulating f32 in PSUM.
  - Instance-norm stats: group 0 via DVE bn_stats, groups 1-3 via ScalarE
    Square/Identity activations with free-dim accumulators; scale/bias math
    on DVE; instnorm+ReLU apply fused into one ScalarE activation writing
    the next layer's fp16 X interior (pads stay zero).
  - Readout: center 2x2 mean (folded into fc weights) + fc matmul (f32r).

Top-k threshold for the pruned graph weight is computed on host
(np.partition) -- weight preprocessing of a replicated input.
"""
import numpy as np

import concourse.bass as bass
import concourse.tile as tile
from concourse import bacc, mybir
from concourse.bass_utils import run_bass_kernel_spmd

F32 = mybir.dt.float32
F32R = mybir.dt.float32r
FP16 = mybir.dt.float16
AF = mybir.ActivationFunctionType
ALU = mybir.AluOpType

N_CORES = 8
B = 32
BPC = B // N_CORES          # 4 samples per core
DIM = 512
DS = 128
FEAT = 256
LAYERS = 8
IMG = 128
OUT = 1000
EPS = 1e-5
HALF = IMG // 4 // 2 - 1    # 15
PRUNE = 0.9
NPX = 1024.0                # pixels per instance-norm map

# --- fp16 conv1 tensor (early): [128, 384]
W1X_OFF = 0                  # 3 dx-taps x [128,128] for conv1
WC1 = 3 * 128

# --- f32 bn1 tensor: [128, 2]
BN1_OFF = 0
WA = 2

# --- f32r main weight tensor: [128, WB]
FCW_OFF = 0                  # 2 kg x [128, 1000] fc lhsT (x0.25 pooled)
FCB_OFF = FCW_OFF + 2 * 1000  # [128, 8] fc bias chunks
BN2_OFF = FCB_OFF + 8        # [128, 2]
NGB_OFF = BN2_OFF + 2        # [128, 12] per og: gamma, beta, gamma*1024
WB = NGB_OFF + 12

# --- fp16 weight tensor: [128, WH] (loaded during the BN1 stall)
W2D_OFF = 0                  # 9 taps x [128,128] diag-dup for conv2
W3_OFF = W2D_OFF + 9 * 128   # [128,128] conv3 (w3 stacked twice on K)
WDW_OFF = W3_OFF + 128       # 4 groups x 9 taps x [128,128] diag
WMIX_OFF = WDW_OFF + 36 * 128  # 4 kg x [128, 512] = w_eff.T blocks
WH = WMIX_OFF + 4 * 512

PE_G = (0, 1)               # depthwise groups on PE
DVE_G = (2, 3)              # depthwise groups on DVE + accum DMA

XP_BUFS = 18

TAPS = [(1, 1), (0, 0), (0, 1), (0, 2), (1, 0), (1, 2), (2, 0), (2, 1), (2, 2)]


def build_nc():
    nc = bacc.Bacc(num_devices=N_CORES)
    x4 = nc.dram_tensor("x4", [BPC, 3, IMG, IMG], FP16, kind="ExternalInput").ap()
    wc1 = nc.dram_tensor("wc1", [128, WC1], FP16, kind="ExternalInput").ap()
    wa = nc.dram_tensor("wa", [128, WA], F32, kind="ExternalInput").ap()
    wb = nc.dram_tensor("wb", [128, WB], F32R, kind="ExternalInput").ap()
    wh = nc.dram_tensor("wh", [128, WH], FP16, kind="ExternalInput").ap()
    wk = nc.dram_tensor("wk", [128, 36], F32, kind="ExternalInput").ap()
    y4 = nc.dram_tensor("y4", [BPC, OUT], F32, kind="ExternalOutput").ap()

    with tile.TileContext(nc) as tc:
        with (
            tc.tile_pool(name="wp", bufs=1) as wp,
            tc.tile_pool(name="small", bufs=1) as small,
            tc.tile_pool(name="psA", bufs=2, space="PSUM") as psA,
            tc.tile_pool(name="psB", bufs=3, space="PSUM") as psB,
            tc.tile_pool(name="dram", bufs=1, space="DRAM") as dram,
        ):
            wc1_sb = wp.tile([128, WC1], FP16)
            nc.sync.dma_start(out=wc1_sb, in_=wc1)
            wa_sb = wp.tile([128, WA], F32)
            nc.sync.dma_start(out=wa_sb, in_=wa)
            wa32 = wa_sb
            # wh/wb/wk tiles are allocated here but their loads are emitted
            # later (after conv1) so they don't delay the im2col DMAs.
            wb_sb = wp.tile([128, WB], F32R)
            wb32 = wb_sb.bitcast(F32)
            wh_sb = wp.tile([128, WH], FP16)
            wk_sb = wp.tile([128, 36], F32)

            z16 = small.tile([128, 130], FP16)
            nc.vector.memset(z16, 0.0)
            zr = z16
            eps_t = small.tile([128, 1], F32)
            nc.vector.memset(eps_t, EPS)
            epsN_t = small.tile([128, 1], F32)
            nc.vector.memset(epsN_t, EPS * NPX * NPX)

            h3 = small.tile([128, 4096], F32)

            # ---------------- downsample ----------------
            with tc.tile_pool(name="ds1", bufs=1) as ds1:
                # im2col9: partition p = 32*s + 3*dy + c ; free = (oy 64, ix' 130)
                im9 = ds1.tile([128, 64 * 130], FP16)
                im9r = im9.rearrange("p (y x) -> p y x", y=64, x=130)
                for xc in (0, 129):
                    im9_pads = bass.AP(tensor=im9.tensor,
                                       offset=im9.offset + xc,
                                       ap=[im9.ap[0], [130, 64]])
                    nc.vector.memset(im9_pads, 0.0)
                # zero row y=0 across all partitions (dy!=0 ranges get
                # overwritten by the x DMAs below)
                nc.vector.memset(im9r[:, 0:1, :], 0.0)
                x4r = x4.rearrange("s c (y2 two) x -> s c y2 two x", two=2)
                for s in (0, 2, 1, 3):
                    for dy in range(3):
                        p0 = 64 * (s % 2) + 27 * (s // 2) + 3 * dy
                        if dy == 0:
                            nc.sync.dma_start(
                                out=im9r[p0:p0 + 3, 1:64, 1:129],
                                in_=x4r[s, :, 0:63, 1, :])

                        elif dy == 1:
                            nc.sync.dma_start(
                                out=im9r[p0:p0 + 3, :, 1:129],
                                in_=x4r[s, :, :, 0, :])
                        else:
                            nc.sync.dma_start(
                                out=im9r[p0:p0 + 3, :, 1:129],
                                in_=x4r[s, :, :, 1, :])

                # conv1: out h1 [128 = 64*(s//2)+ch, (s%2, oy 64, ox 64)]
                h1 = ds1.tile([128, 8192], F32)
                h1r = h1.rearrange("p (sh y x) -> p sh y x", sh=2, y=64, x=64)
                im9x = im9.rearrange("p (y x2 two) -> p y x2 two", x2=65, two=2)
                for q in range(2):
                    for yb in range(4):
                        for h in range(2):
                            pc1 = psA.tile([128, 512], F32, tag="a",
                                           name="pc1")
                            pc1r = pc1.rearrange("p (y x) -> p y x", y=8, x=64)
                            oy0 = yb * 16 + h * 8
                            for di, dx in enumerate([1, 0, 2]):
                                rhs = im9x[64 * q:64 * q + 54, oy0:oy0 + 8,
                                           dx // 2:dx // 2 + 64, dx % 2]
                                lhsT = wc1_sb[64 * q:64 * q + 54,
                                              W1X_OFF + di_col(dx) * 128:
                                              W1X_OFF + di_col(dx) * 128 + 128]
                                nc.tensor.matmul(pc1r, lhsT, rhs,
                                                 start=(di == 0), stop=(di == 2),
                                                 tile_position=(64 * q, 0))
                            if (q + yb + h) % 2 == 0:
                                nc.scalar.copy(
                                    out=h1r[:, q, oy0:oy0 + 8, :], in_=pc1)
                            else:
                                nc.vector.tensor_copy(
                                    out=h1r[:, q, oy0:oy0 + 8, :], in_=pc1)

                # deferred big weight loads (overlap the BN1 collective)
                nc.sync.dma_start(out=wh_sb, in_=wh)
                nc.sync.dma_start(out=wb_sb, in_=wb)
                nc.sync.dma_start(out=wk_sb, in_=wk)

                # BN1 partial stats -> global sums via AllGather
                st1 = small.tile([128, 16, 6], F32)
                for i in range(16):
                    nc.vector.bn_stats(out=st1[:, i, :],
                                       in_=h1[:, i * 512:(i + 1) * 512])
                mv1 = small.tile([128, 2], F32)
                nc.vector.bn_aggr(out=mv1, in_=st1)
                sums1 = small.tile([128, 2], F32)
                tmp1 = small.tile([128, 1], F32)
                nc.vector.tensor_scalar_mul(out=sums1[:, 0:1], in0=mv1[:, 0:1],
                                            scalar1=8192.0)
                nc.vector.tensor_mul(out=tmp1, in0=mv1[:, 0:1], in1=mv1[:, 0:1])
                nc.vector.tensor_add(out=tmp1, in0=tmp1, in1=mv1[:, 1:2])
                nc.vector.tensor_scalar_mul(out=sums1[:, 1:2], in0=tmp1,
                                            scalar1=8192.0)
                red1 = _allgather_sum(nc, tc, dram, small, sums1, "bn1",
                                      fold_halves=True)
                s1t1 = small.tile([128, 2], F32)
                _bn_scale_bias(nc, s1t1, red1, wa32, BN1_OFF, 131072.0,
                               eps_t, small, rows=128)

                # apply BN1 + relu -> h1n (f32r), x padded to 66
                h1n = ds1.tile([128, 2 * 64 * 66], FP16)
                h1nr3 = h1n.rearrange("p (sh y x) -> p sh y x",
                                      sh=2, y=64, x=66)
                for sh in range(2):
                    for xc in (0, 65):
                        h1n_pads = bass.AP(tensor=h1n.tensor,
                                           offset=h1n.offset + 4224 * sh + xc,
                                           ap=[h1n.ap[0], [66, 64]])
                        nc.vector.memset(h1n_pads, 0.0)
                h1r4 = h1.rearrange("p (sh y x) -> p sh y x", sh=2, y=64, x=64)
                for sh in range(2):
                    for y0, y1 in ((0, 34), (34, 64)):
                        nc.scalar.activation(out=h1nr3[:, sh, y0:y1, 1:65],
                                             in_=h1r4[:, sh, y0:y1, :],
                                             func=AF.Relu,
                                             scale=s1t1[:, 0:1], bias=s1t1[:, 1:2])

                # conv2: depthwise 3x3 stride 2 -> d2 [128, (sh, 32, 32)]
                h1nr = h1n.rearrange(
                    "p (sh y2 ty x2 tx) -> p sh y2 ty x2 tx",
                    sh=2, y2=32, ty=2, x2=33, tx=2)
                d2 = ds1.tile([128, 2048], FP16)
                for sh in range(2):
                    pd2 = psB.tile([128, 1024], F32, tag="b", name="pd2")
                    pd2r = pd2.rearrange("p (h y x) -> p h y x", h=2, y=16, x=32)
                    for h in range(2):
                        for ti, (dy, dx) in enumerate(TAPS):
                            oy0, oy1 = _clip(h * 16, h * 16 + 16,
                                             1 if dy == 0 else 0, 32)
                            if dy == 1:
                                ys, par = oy0, 0
                            elif dy == 0:
                                ys, par = oy0 - 1, 1
                            else:
                                ys, par = oy0, 1
                            rhs = h1nr[:, sh, ys:ys + (oy1 - oy0), par,
                                       dx // 2:dx // 2 + 32, dx % 2]
                            outp = pd2r[:, h, oy0 - h * 16:oy1 - h * 16, :]
                            t = TAPS.index((dy, dx))
                            nc.tensor.matmul(
                                outp, wh_sb[:, W2D_OFF + t * 128:
                                            W2D_OFF + t * 128 + 128], rhs,
                                start=(ti == 0), stop=(ti == len(TAPS) - 1))
                    nc.scalar.copy(out=d2[:, sh * 1024:(sh + 1) * 1024], in_=pd2)

                # conv3: 1x1, 64 -> 128 ; h3 [128=outc, (s, 1024px)]
                for a in range(2):
                    for nb in range(4):
                        pc3 = psA.tile([128, 512], F32, tag="a",
                                       name=f"pc3_{a}_{nb}")
                        nc.tensor.matmul(
                            pc3,
                            wh_sb[64 * a:64 * a + 64, W3_OFF:W3_OFF + 128],
                            d2[64 * a:64 * a + 64, nb * 512:(nb + 1) * 512],
                            start=True, stop=True)
                        s_full = 2 * a + nb // 2
                        dst = h3[:, s_full * 1024 + (nb % 2) * 512:
                                 s_full * 1024 + (nb % 2) * 512 + 512]
                        if nb % 2 == 0:
                            nc.scalar.copy(out=dst, in_=pc3)
                        else:
                            nc.vector.tensor_copy(out=dst, in_=pc3)

                # BN2 stats + allgather
                st2 = small.tile([128, 8, 6], F32)
                for i in range(8):
                    nc.vector.bn_stats(out=st2[:, i, :],
                                       in_=h3[:, i * 512:(i + 1) * 512])
                mv2 = small.tile([128, 2], F32)
                nc.vector.bn_aggr(out=mv2, in_=st2)
                sums2 = small.tile([128, 2], F32)
                tmp2 = small.tile([128, 1], F32)
                nc.vector.tensor_scalar_mul(out=sums2[:, 0:1], in0=mv2[:, 0:1],
                                            scalar1=4096.0)
                nc.vector.tensor_mul(out=tmp2, in0=mv2[:, 0:1], in1=mv2[:, 0:1])
                nc.vector.tensor_add(out=tmp2, in0=tmp2, in1=mv2[:, 1:2])
                nc.vector.tensor_scalar_mul(out=sums2[:, 1:2], in0=tmp2,
                                            scalar1=4096.0)
                red2 = _allgather_sum(nc, tc, dram, small, sums2, "bn2")
                s2t2 = small.tile([128, 2], F32)
                _bn_scale_bias(nc, s2t2, red2, wb32, BN2_OFF, 32768.0,
                               eps_t, small, rows=128)

            # ---------------- main loop ----------------
            with (
                tc.tile_pool(name="xp", bufs=XP_BUFS) as xp,
                tc.tile_pool(name="sa", bufs=6) as sa_pool,
                tc.tile_pool(name="sb", bufs=4) as sb_pool,
                tc.tile_pool(name="sc", bufs=4) as sc_pool,
                tc.tile_pool(name="dp", bufs=8) as dp,
                tc.tile_pool(name="stp", bufs=8) as stp,
            ):
                def new_x_tile(name):
                    # pads (rows/cols 0 and 33) of every xp slot were zeroed
                    # once below; applies only write the interior.
                    return xp.tile([128, 34 * 34], FP16, tag="X", name=name)

                _dummies = []
                for i in range(XP_BUFS):
                    zt = xp.tile([128, 34 * 34], FP16, tag="X", name=f"xz{i}")
                    rows = bass.AP(tensor=zt.tensor, offset=zt.offset,
                                   ap=[zt.ap[0], [33 * 34, 2], [1, 34]])
                    cols = bass.AP(tensor=zt.tensor, offset=zt.offset,
                                   ap=[zt.ap[0], [34, 34], [33, 2]])
                    nc.vector.memset(rows, 0.0)
                    nc.vector.memset(cols, 0.0)
                    _dummies.append(zt)
                _pad_scratch = small.tile([128, 1], FP16)
                for zt in _dummies:
                    nc.vector.tensor_copy(out=_pad_scratch, in_=zt[:, 0:1])

                junk1 = small.tile([128, 1024], FP16)
                junk2 = small.tile([128, 1024], FP16)

                h3r = h3.rearrange("p (s y x) -> p s y x", s=4, y=32, x=32)
                Xcur = {}
                for s in range(BPC):
                    xt = new_x_tile(f"X1_0_{s}")
                    xtr = xt.rearrange("p (y x) -> p y x", y=34, x=34)
                    nc.scalar.activation(out=xtr[:, 1:33, 1:33],
                                         in_=h3r[:, s, :, :],
                                         func=AF.Relu,
                                         scale=s2t2[:, 0:1], bias=s2t2[:, 1:2])
                    Xcur[(0, s)] = xt

                pooled_in = small.tile([128, 2, 4, 4], F32)

                for L in range(1, LAYERS + 1):
                    gs_in = sorted({g for (g, _s) in Xcur})
                    mgs = [2, 3] if L == LAYERS else [0, 1, 2, 3]
                    Xnext = {}
                    Dall = {}

                    def emit_dw(s, L=L, gs_in=gs_in, Dall=Dall, Xcur=Xcur):
                        for g in gs_in:
                            Xr = Xcur[(g, s)].rearrange("p (y x) -> p y x",
                                                        y=34, x=34)
                            if (g in PE_G) if L > 1 else (s < 3):
                                dD = dp.tile([128, 1024], FP16, tag="D",
                                             name=f"D{L}_{s}_{g}")
                                for h in range(2):
                                    pdw = psA.tile([128, 512], F32, tag="a",
                                                   name=f"pdw{L}_{s}_{g}_{h}")
                                    pdwr = pdw.rearrange("p (y x) -> p y x",
                                                         y=16, x=32)
                                    for ti, (dy, dx) in enumerate(TAPS):
                                        rhs = Xr[:, h * 16 + dy:h * 16 + 16 + dy,
                                                 dx:dx + 32]
                                        nc.tensor.matmul(
                                            pdwr,
                                            wh_sb[:, WDW_OFF + (g * 9 + ti) * 128:
                                                  WDW_OFF + (g * 9 + ti) * 128 + 128],
                                            rhs,
                                            start=(ti == 0),
                                            stop=(ti == len(TAPS) - 1))
                                    nc.scalar.copy(
                                        out=dD[:, h * 512:h * 512 + 512],
                                        in_=pdw)
                                Dall[(g, s)] = dD
                            else:
                                # DVE products into plane stacks
                                sA = sa_pool.tile([128, 4, 1024], FP16, tag="A",
                                                  name=f"A{L}_{s}_{g}")
                                sB = sb_pool.tile([128, 4, 1024], FP16, tag="B",
                                                  name=f"B{L}_{s}_{g}")
                                sC = sc_pool.tile([128, 1024], FP16, tag="C",
                                                  name=f"C{L}_{s}_{g}")
                                for ti, (dy, dx) in enumerate(TAPS):
                                    if ti < 4:
                                        dst = sA[:, ti, :]
                                    elif ti < 8:
                                        dst = sB[:, ti - 4, :]
                                    else:
                                        dst = sC
                                    nc.vector.tensor_scalar_mul(
                                        out=dst.rearrange("p (y x) -> p y x",
                                                          y=32, x=32),
                                        in0=Xr[:, dy:dy + 32, dx:dx + 32],
                                        scalar1=wk_sb[:, g * 9 + ti:
                                                      g * 9 + ti + 1])
                                # 2 accumulate-DMAs: A[0:2] += B[0:2]; A[2:4] += B[2:4]
                                nc.gpsimd.dma_start(out=sA[:, 0:2, :],
                                                    in_=sB[:, 0:2, :],
                                                    accum_op=ALU.add)
                                nc.gpsimd.dma_start(out=sA[:, 2:4, :],
                                                    in_=sB[:, 2:4, :],
                                                    accum_op=ALU.add)
                                # 3 DVE adds: A[0:2] += A[2:4]; A0 += A1; A0 += C
                                nc.vector.tensor_add(out=sA[:, 0:2, :],
                                                     in0=sA[:, 0:2, :],
                                                     in1=sA[:, 2:4, :])
                                nc.vector.tensor_add(out=sA[:, 0, :],
                                                     in0=sA[:, 0, :],
                                                     in1=sA[:, 1, :])
                                eng = (nc.vector if L in (1, LAYERS)
                                       else nc.gpsimd)
                                eng.tensor_add(out=sA[:, 0, :],
                                               in0=sA[:, 0, :],
                                               in1=sC)
                                Dall[(g, s)] = sA

                    def emit_mix(s, L=L, gs_in=gs_in, mgs=mgs, Dall=Dall,
                                 Xnext=Xnext):
                        for mg in mgs:
                            pm = psB.tile([128, 1024], F32, tag="b",
                                          name=f"pm{L}_{s}_{mg}")
                            for h in range(2):
                                for ki, kg in enumerate(gs_in):
                                    dD = Dall[(kg, s)]
                                    rhs = (dD[:, 0, h * 512:h * 512 + 512]
                                           if len(dD.shape) == 3
                                           else dD[:, h * 512:h * 512 + 512])
                                    nc.tensor.matmul(
                                        pm[:, h * 512:h * 512 + 512],
                                        wh_sb[:, WMIX_OFF + kg * 512 + mg * 128:
                                              WMIX_OFF + kg * 512 + mg * 128 + 128],
                                        rhs,
                                        start=(ki == 0),
                                        stop=(ki == len(gs_in) - 1))
                            sc = stp.tile([128, 1], F32, tag="sc")
                            tt = stp.tile([128, 1], F32, tag="tt")
                            if mg in (0, 1) or s == 0 or L == LAYERS:
                                st = stp.tile([128, 2, 6], F32, tag="st")
                                nc.vector.bn_stats(out=st[:, 0, :],
                                                   in_=pm[:, 0:512])
                                nc.vector.bn_stats(out=st[:, 1, :],
                                                   in_=pm[:, 512:1024])
                                mv = stp.tile([128, 2], F32, tag="mv")
                                nc.vector.bn_aggr(out=mv, in_=st)
                                # sc = gamma * rsqrt(var+eps); tt = beta - mean*sc
                                nc.scalar.activation(out=sc, in_=mv[:, 1:2],
                                                     func=AF.Sqrt, bias=eps_t)
                                nc.vector.reciprocal(out=sc, in_=sc)
                                nc.vector.tensor_scalar_mul(
                                    out=sc, in0=sc,
                                    scalar1=wb32[:, NGB_OFF + 3 * mg:
                                                 NGB_OFF + 3 * mg + 1])
                                nc.vector.tensor_mul(out=tt, in0=mv[:, 0:1],
                                                     in1=sc)
                                nc.vector.tensor_scalar(
                                    out=tt, in0=tt, scalar1=-1.0,
                                    scalar2=wb32[:, NGB_OFF + 3 * mg + 1:
                                                 NGB_OFF + 3 * mg + 2],
                                    op0=ALU.mult, op1=ALU.add)
                            else:
                                # Act-engine stats: S1 = sum(pm), S2 = sum(pm^2)
                                s1 = stp.tile([128, 1], F32, tag="s1")
                                s2 = stp.tile([128, 1], F32, tag="s2")
                                nc.scalar.activation(out=junk1, in_=pm,
                                                     func=AF.Square,
                                                     accum_out=s2)
                                nc.scalar.activation(out=junk2, in_=pm,
                                                     func=AF.Identity,
                                                     accum_out=s1)
                                # V = N*S2 - S1^2 ; sc = gamma*N/sqrt(V+N^2*eps)
                                v = stp.tile([128, 1], F32, tag="v")
                                nc.vector.tensor_mul(out=v, in0=s1, in1=s1)
                                nc.vector.tensor_scalar(
                                    out=v, in0=s2, scalar1=NPX, scalar2=v,
                                    op0=ALU.mult, op1=ALU.subtract)
                                nc.scalar.activation(out=sc, in_=v,
                                                     func=AF.Sqrt, bias=epsN_t)
                                nc.vector.reciprocal(out=sc, in_=sc)
                                nc.vector.tensor_scalar_mul(
                                    out=sc, in0=sc,
                                    scalar1=wb32[:, NGB_OFF + 3 * mg + 2:
                                                 NGB_OFF + 3 * mg + 3])
                                # tt = beta - (S1/N)*sc
                                nc.vector.tensor_mul(out=tt, in0=s1, in1=sc)
                                nc.vector.tensor_scalar(
                                    out=tt, in0=tt, scalar1=-1.0 / NPX,
                                    scalar2=wb32[:, NGB_OFF + 3 * mg + 1:
                                                 NGB_OFF + 3 * mg + 2],
                                    op0=ALU.mult, op1=ALU.add)
                            if L < LAYERS:
                                xt = new_x_tile(f"X{L + 1}_{mg}_{s}")
                                Xnext[(mg, s)] = xt
                                xtr = xt.rearrange("p (y x) -> p y x",
                                                   y=34, x=34)
                                pmr2 = pm.rearrange("p (y x) -> p y x",
                                                    y=32, x=32)
                                nc.scalar.activation(
                                    out=xtr[:, 1:33, 1:33], in_=pmr2,
                                    func=AF.Relu, scale=sc, bias=tt)
                            else:
                                pmr = pm.rearrange("p (h y x) -> p h y x",
                                                   h=2, y=16, x=32)
                                nc.scalar.activation(
                                    out=pooled_in[:, mg - 2, s, :],
                                    in_=pmr[:, 0, HALF - 1:HALF + 1,
                                            HALF - 1:HALF + 1],
                                    func=AF.Identity, scale=sc, bias=tt)

                    # software-pipelined emission: dw(0) dw(1) mix(0) dw(2)
                    # mix(1) dw(3) mix(2) mix(3).  Last layer: all dw first
                    # (mix is half-sized and cannot hide the DVE chains).
                    if L == LAYERS:
                        emit_dw(0)
                        emit_dw(1)
                        emit_dw(2)
                        emit_mix(0)
                        emit_dw(3)
                        emit_mix(1)
                        emit_mix(2)
                        emit_mix(3)
                    else:
                        emit_dw(0)
                        emit_dw(1)
                        emit_mix(0)
                        emit_dw(2)
                        emit_mix(1)
                        emit_dw(3)
                        emit_mix(2)
                        emit_mix(3)
                    Xcur = Xnext

                # ---------------- readout ----------------
                tadd = small.tile([128, 2, 4], F32)
                tadd2 = small.tile([128, 2, 4], F32)
                pooled = small.tile([128, 2, 4], F32R)
                nc.vector.tensor_add(out=tadd, in0=pooled_in[:, :, :, 0],
                                     in1=pooled_in[:, :, :, 1])
                nc.vector.tensor_add(out=tadd2, in0=pooled_in[:, :, :, 2],
                                     in1=pooled_in[:, :, :, 3])
                nc.vector.tensor_add(out=pooled, in0=tadd, in1=tadd2)
                y_sb = small.tile([128, 8, 4], F32)
                for mo in range(8):
                    mlen = 128 if mo < 7 else OUT - 7 * 128
                    pf = psA.tile([128, 512], F32, tag="a", name=f"pf{mo}")
                    for kgi in range(2):
                        nc.tensor.matmul(
                            pf[0:mlen, 0:4],
                            wb_sb[:, FCW_OFF + kgi * 1000 + mo * 128:
                                  FCW_OFF + kgi * 1000 + mo * 128 + mlen],
                            pooled[:, kgi, :],
                            start=(kgi == 0), stop=(kgi == 1))
                    nc.scalar.activation(
                        out=y_sb[0:mlen, mo, :], in_=pf[0:mlen, 0:4],
                        func=AF.Identity,
                        bias=wb32[0:mlen, FCB_OFF + mo:FCB_OFF + mo + 1],
                        scale=1.0)
                    dst = bass.AP(tensor=y4.tensor, offset=128 * mo,
                                  ap=[[1, mlen], [OUT, BPC]])
                    nc.sync.dma_start(out=dst, in_=y_sb[0:mlen, mo, :])

    nc.finalize()
    return nc


def _allgather_sum(nc, tc, dram, small, sums, name, fold_halves=False):
    """AllGather the [128,2] partial sums across 8 cores and sum locally.
    With fold_halves, partition halves (64a+ch) are also summed and the
    result is duplicated to all 128 partitions.  Returns [128,2] SBUF."""
    cc_in = dram.tile([128, 2], F32, name=f"{name}_in")
    cc_out = dram.tile([N_CORES, 256], F32, name=f"{name}_out")
    nc.sync.dma_start(out=cc_in, in_=sums)
    nc.gpsimd.collective_compute(
        "AllGather", ALU.bypass,
        replica_groups=[list(range(N_CORES))],
        ins=[cc_in.opt()], outs=[cc_out.opt()])
    if fold_halves:
        # partition p reads (core r, half h) slot value [(p%64)*2 + j]
        red = small.tile([128, 16, 2], F32, name=f"{name}_red")
        for half in range(2):
            nc.sync.dma_start(
                out=red[64 * half:64 * half + 64, :, :],
                in_=bass.AP(tensor=cc_out.tensor, offset=cc_out.offset,
                            ap=[[2, 64], [128, 16], [1, 2]]))
        nc.vector.tensor_add(out=red[:, 0:8, :], in0=red[:, 0:8, :],
                             in1=red[:, 8:16, :])
    else:
        red = small.tile([128, 16, 2], F32, name=f"{name}_red")
        nc.sync.dma_start(
            out=red[:, 0:8, :],
            in_=bass.AP(tensor=cc_out.tensor, offset=cc_out.offset,
                        ap=[[2, 128], [256, N_CORES], [1, 2]]))
    nc.vector.tensor_add(out=red[:, 0:4, :], in0=red[:, 0:4, :],
                         in1=red[:, 4:8, :])
    nc.vector.tensor_add(out=red[:, 0:2, :], in0=red[:, 0:2, :],
                         in1=red[:, 2:4, :])
    nc.vector.tensor_add(out=red[:, 0, :], in0=red[:, 0, :],
                         in1=red[:, 1, :])
    return red[:, 0, :]


def di_col(dx):
    return {1: 0, 0: 1, 2: 2}[dx]


def _clip(lo, hi, lo2, hi2):
    return max(lo, lo2), min(hi, hi2)


def _bn_scale_bias(nc, out_st, sums, w32, gb_off, n_tot, eps_t, pool, rows):
    """out_st[:rows, 0] = gamma*rsqrt(var+eps); out_st[:rows, 1] = beta - mu*scale."""
    r = slice(0, rows)
    mu = pool.tile([128, 1], F32, name=f"mu{gb_off}")
    ex2 = pool.tile([128, 1], F32, name=f"ex2{gb_off}")
    var = pool.tile([128, 1], F32, name=f"var{gb_off}")
    nc.vector.tensor_scalar_mul(out=mu[r], in0=sums[r, 0:1], scalar1=1.0 / n_tot)
    nc.vector.tensor_scalar_mul(out=ex2[r], in0=sums[r, 1:2], scalar1=1.0 / n_tot)
    nc.vector.tensor_mul(out=var[r], in0=mu[r], in1=mu[r])
    nc.vector.tensor_sub(out=var[r], in0=ex2[r], in1=var[r])
    nc.scalar.activation(out=var[r], in_=var[r], func=AF.Sqrt, bias=eps_t[r])
    nc.vector.reciprocal(out=var[r], in_=var[r])
    nc.vector.tensor_scalar_mul(out=out_st[r, 0:1], in0=var[r],
                                scalar1=w32[r, gb_off:gb_off + 1])
    nc.vector.tensor_mul(out=mu[r], in0=mu[r], in1=out_st[r, 0:1])
    nc.vector.tensor_scalar(out=out_st[r, 1:2], in0=mu[r], scalar1=-1.0,
                            scalar2=w32[r, gb_off + 1:gb_off + 2],
                            op0=ALU.mult, op1=ALU.add)


def _pack_weights(ds_w1, ds_w2, ds_w3, conv_w, graph_w, fc_w, fc_b,
                  bn1_g, bn1_b, bn2_g, bn2_b, norm_g, norm_b):
    wc1 = np.zeros((128, WC1), np.float16)
    wa = np.zeros((128, WA), np.float32)
    wb = np.zeros((128, WB), np.float32)
    wh = np.zeros((128, WH), np.float16)
    wk = np.zeros((128, 36), np.float32)
    # pruned graph weight
    k = int((1.0 - PRUNE) * DIM * DIM)
    a = np.abs(graph_w).ravel()
    thresh = np.partition(a, -k)[-k]
    w_eff = np.where(np.abs(graph_w) >= thresh, graph_w, 0.0).astype(np.float32)
    # conv1 taps, paired block-diag
    for dx in range(3):
        dc = di_col(dx)
        blk = np.zeros((128, 128), np.float32)
        for qq in range(2):
            for aa in range(2):
                for dy in range(3):
                    for c in range(3):
                        blk[64 * qq + 27 * aa + 3 * dy + c,
                            64 * aa:64 * aa + 64] = ds_w1[:, c, dy, dx]
        wc1[:, W1X_OFF + dc * 128:W1X_OFF + (dc + 1) * 128] = blk.astype(np.float16)
    wa[0:64, BN1_OFF] = bn1_g
    wa[64:128, BN1_OFF] = bn1_g
    wa[0:64, BN1_OFF + 1] = bn1_b
    wa[64:128, BN1_OFF + 1] = bn1_b
    # conv2 diag-dup taps (fp16)
    for t, (dy, dx) in enumerate(TAPS):
        blk = np.zeros((128, 128), np.float16)
        d = ds_w2[:, 0, dy, dx].astype(np.float16)
        for aa in range(2):
            idx = np.arange(64)
            blk[64 * aa + idx, 64 * aa + idx] = d
        wh[:, W2D_OFF + t * 128:W2D_OFF + (t + 1) * 128] = blk
    # conv3 (fp16)
    w3 = ds_w3[:, :, 0, 0].astype(np.float16)
    wh[0:64, W3_OFF:W3_OFF + 128] = w3.T
    wh[64:128, W3_OFF:W3_OFF + 128] = w3.T
    # main dw diag taps (fp16) + tap scalar columns
    for g in range(4):
        for t, (dy, dx) in enumerate(TAPS):
            blk = np.zeros((128, 128), np.float16)
            idx = np.arange(128)
            kv = conv_w[g * 128:(g + 1) * 128, 0, dy, dx]
            blk[idx, idx] = kv.astype(np.float16)
            off = WDW_OFF + (g * 9 + t) * 128
            wh[:, off:off + 128] = blk
            wk[:, g * 9 + t] = kv
    # mix (fp16)
    weT = w_eff.T
    for kg in range(4):
        wh[:, WMIX_OFF + kg * 512:WMIX_OFF + (kg + 1) * 512] = \
            weT[kg * 128:(kg + 1) * 128, :].astype(np.float16)
    # fc (x0.25 for the 2x2 mean)
    for kg in range(2):
        wb[:, FCW_OFF + kg * 1000:FCW_OFF + (kg + 1) * 1000] = \
            0.25 * fc_w[:, kg * 128:(kg + 1) * 128].T
    fcb = np.zeros((128, 8), np.float32)
    fb = np.zeros(1024, np.float32)
    fb[:OUT] = fc_b
    fcb[:, :] = fb.reshape(8, 128).T
    wb[:, FCB_OFF:FCB_OFF + 8] = fcb
    wb[:, BN2_OFF] = bn2_g
    wb[:, BN2_OFF + 1] = bn2_b
    for g in range(4):
        wb[:, NGB_OFF + 3 * g] = norm_g[g * 128:(g + 1) * 128]
        wb[:, NGB_OFF + 3 * g + 1] = norm_b[g * 128:(g + 1) * 128]
        wb[:, NGB_OFF + 3 * g + 2] = norm_g[g * 128:(g + 1) * 128] * NPX
    return wc1, wa, wb, wh, wk


_nc_cache = None
last_results = None


def kernel(**inputs):
    global _nc_cache, last_results
    inputs = {k: np.asarray(v, np.float32) for k, v in inputs.items()}
    wc1, wa, wb, wh, wk = _pack_weights(
        inputs["ds_w1"], inputs["ds_w2"], inputs["ds_w3"], inputs["conv_w"],
        inputs["graph_w"], inputs["fc_w"], inputs["fc_b"],
        inputs["bn1_g"], inputs["bn1_b"], inputs["bn2_g"], inputs["bn2_b"],
        inputs["norm_g"], inputs["norm_b"])
    x = inputs["x"]
    if _nc_cache is None:
        _nc_cache = build_nc()
    nc = _nc_cache
    x16 = x.astype(np.float16)
    in_maps = [{"x4": np.ascontiguousarray(x16[c * BPC:(c + 1) * BPC]),
                "wc1": wc1, "wa": wa, "wb": wb, "wh": wh, "wk": wk}
               for c in range(N_CORES)]
    res = run_bass_kernel_spmd(nc, in_maps, core_ids=list(range(N_CORES)))
    last_results = res
    return np.concatenate([res.results[c]["y4"] for c in range(N_CORES)], axis=0)


# revision 32
# speedup vs baseline: 1.0090x; 1.0090x over previous
"""Trainium2 Bass kernel for nn_DiscreteTimeNeuralGraph.

Strategy (8 NeuronCores, batch-parallel, 4 samples/core):
  - Downsample path on PE (f32r); BatchNorm batch stats via per-core partial
    sums + AllGather (cheaper than AllReduce in this stack) + local sum.
  - Main loop in fp16: X maps padded to 34x34 so all 9 depthwise taps are
    full-range.  Depthwise conv split across engines:
      groups 0,1 -> PE as 9 rect diagonal matmuls/half (psum), Pool copies
                    psum -> fp16 D tiles;
      groups 2,3 -> DVE: 9 per-channel-scalar products (fp16, 4x DVE mode)
                    into plane stacks, then 2 accumulate-DMAs (Pool SWDGE)
                    + 3 DVE adds to reduce the 9 planes.
    Channel mix (pruned 512x512, dense fp16) as K-blocked matmuls on PE
    accumulating f32 in PSUM.
  - Instance-norm stats: group 0 via DVE bn_stats, groups 1-3 via ScalarE
    Square/Identity activations with free-dim accumulators; scale/bias math
    on DVE; instnorm+ReLU apply fused into one ScalarE activation writing
    the next layer's fp16 X interior (pads stay zero).
  - Readout: center 2x2 mean (folded into fc weights) + fc matmul (f32r).

Top-k threshold for the pruned graph weight is computed on host
(np.partition) -- weight preprocessing of a replicated input.
"""
import time

import numpy as np

import concourse.bass as bass
import concourse.tile as tile
from concourse import bacc, mybir
from concourse.bass_utils import run_bass_kernel_spmd

F32 = mybir.dt.float32
F32R = mybir.dt.float32r
FP16 = mybir.dt.float16
AF = mybir.ActivationFunctionType
ALU = mybir.AluOpType

N_CORES = 8
B = 32
BPC = B // N_CORES          # 4 samples per core
DIM = 512
DS = 128
FEAT = 256
LAYERS = 8
IMG = 128
OUT = 1000
EPS = 1e-5
HALF = IMG // 4 // 2 - 1    # 15
PRUNE = 0.9
NPX = 1024.0                # pixels per instance-norm map

# --- fp16 conv1 tensor (early): [128, 384]
W1X_OFF = 0                  # 3 dx-taps x [128,128] for conv1
WC1 = 3 * 128

# --- f32 bn1 tensor: [128, 2]
BN1_OFF = 0
WA = 2

# --- f32r main weight tensor: [128, WB]
FCW_OFF = 0                  # 2 kg x [128, 1000] fc lhsT (x0.25 pooled)
FCB_OFF = FCW_OFF + 2 * 1000  # [128, 8] fc bias chunks
BN2_OFF = FCB_OFF + 8        # [128, 2]
NGB_OFF = BN2_OFF + 2        # [128, 12] per og: gamma, beta, gamma*1024
WB = NGB_OFF + 12

# --- fp16 weight tensor: [128, WH] (loaded during the BN1 stall)
W2D_OFF = 0                  # 9 taps x [128,128] diag-dup for conv2
W3_OFF = W2D_OFF + 9 * 128   # [128,128] conv3 (w3 stacked twice on K)
WDW_OFF = W3_OFF + 128       # 4 groups x 9 taps x [128,128] diag
WMIX_OFF = WDW_OFF + 36 * 128  # 4 kg x [128, 512] = w_eff.T blocks
WH = WMIX_OFF + 4 * 512

PE_G = (0, 1)               # depthwise groups on PE
DVE_G = (2, 3)              # depthwise groups on DVE + accum DMA

XP_BUFS = 18

TAPS = [(1, 1), (0, 0), (0, 1), (0, 2), (1, 0), (1, 2), (2, 0), (2, 1), (2, 2)]


def build_nc():
    nc = bacc.Bacc(num_devices=N_CORES)
    x4 = nc.dram_tensor("x4", [BPC, 3, IMG, IMG], FP16, kind="ExternalInput").ap()
    wc1 = nc.dram_tensor("wc1", [128, WC1], FP16, kind="ExternalInput").ap()
    wa = nc.dram_tensor("wa", [128, WA], F32, kind="ExternalInput").ap()
    wb = nc.dram_tensor("wb", [128, WB], F32R, kind="ExternalInput").ap()
    wh = nc.dram_tensor("wh", [128, WH], FP16, kind="ExternalInput").ap()
    wk = nc.dram_tensor("wk", [128, 36], F32, kind="ExternalInput").ap()
    y4 = nc.dram_tensor("y4", [BPC, OUT], F32, kind="ExternalOutput").ap()

    with tile.TileContext(nc) as tc:
        with (
            tc.tile_pool(name="wp", bufs=1) as wp,
            tc.tile_pool(name="small", bufs=1) as small,
            tc.tile_pool(name="psA", bufs=2, space="PSUM") as psA,
            tc.tile_pool(name="psB", bufs=3, space="PSUM") as psB,
            tc.tile_pool(name="dram", bufs=1, space="DRAM") as dram,
        ):
            wc1_sb = wp.tile([128, WC1], FP16)
            nc.sync.dma_start(out=wc1_sb, in_=wc1)
            wa_sb = wp.tile([128, WA], F32)
            nc.sync.dma_start(out=wa_sb, in_=wa)
            wa32 = wa_sb
            # wh/wb/wk tiles are allocated here but their loads are emitted
            # later (after conv1) so they don't delay the im2col DMAs.
            wb_sb = wp.tile([128, WB], F32R)
            wb32 = wb_sb.bitcast(F32)
            wh_sb = wp.tile([128, WH], FP16)
            wk_sb = wp.tile([128, 36], F32)

            z16 = small.tile([128, 130], FP16)
            nc.vector.memset(z16, 0.0)
            zr = z16
            eps_t = small.tile([128, 1], F32)
            nc.vector.memset(eps_t, EPS)
            epsN_t = small.tile([128, 1], F32)
            nc.vector.memset(epsN_t, EPS * NPX * NPX)

            h3 = small.tile([128, 4096], F32)

            # ---------------- downsample ----------------
            with tc.tile_pool(name="ds1", bufs=1) as ds1:
                # im2col9: partition p = 32*s + 3*dy + c ; free = (oy 64, ix' 130)
                im9 = ds1.tile([128, 64 * 130], FP16)
                im9r = im9.rearrange("p (y x) -> p y x", y=64, x=130)
                for xc in (0, 129):
                    im9_pads = bass.AP(tensor=im9.tensor,
                                       offset=im9.offset + xc,
                                       ap=[im9.ap[0], [130, 64]])
                    nc.vector.memset(im9_pads, 0.0)
                # zero row y=0 across all partitions (dy!=0 ranges get
                # overwritten by the x DMAs below)
                nc.vector.memset(im9r[:, 0:1, :], 0.0)
                x4r = x4.rearrange("s c (y2 two) x -> s c y2 two x", two=2)
                for s in (0, 2, 1, 3):
                    for dy in range(3):
                        p0 = 64 * (s % 2) + 27 * (s // 2) + 3 * dy
                        if dy == 0:
                            nc.sync.dma_start(
                                out=im9r[p0:p0 + 3, 1:64, 1:129],
                                in_=x4r[s, :, 0:63, 1, :])

                        elif dy == 1:
                            nc.sync.dma_start(
                                out=im9r[p0:p0 + 3, :, 1:129],
                                in_=x4r[s, :, :, 0, :])
                        else:
                            nc.sync.dma_start(
                                out=im9r[p0:p0 + 3, :, 1:129],
                                in_=x4r[s, :, :, 1, :])

                # conv1: out h1 [128 = 64*(s//2)+ch, (s%2, oy 64, ox 64)]
                h1 = ds1.tile([128, 8192], F32)
                h1r = h1.rearrange("p (sh y x) -> p sh y x", sh=2, y=64, x=64)
                im9x = im9.rearrange("p (y x2 two) -> p y x2 two", x2=65, two=2)
                for q in range(2):
                    for yb in range(4):
                        for h in range(2):
                            pc1 = psA.tile([128, 512], F32, tag="a",
                                           name="pc1")
                            pc1r = pc1.rearrange("p (y x) -> p y x", y=8, x=64)
                            oy0 = yb * 16 + h * 8
                            for di, dx in enumerate([1, 0, 2]):
                                rhs = im9x[64 * q:64 * q + 54, oy0:oy0 + 8,
                                           dx // 2:dx // 2 + 64, dx % 2]
                                lhsT = wc1_sb[64 * q:64 * q + 54,
                                              W1X_OFF + di_col(dx) * 128:
                                              W1X_OFF + di_col(dx) * 128 + 128]
                                nc.tensor.matmul(pc1r, lhsT, rhs,
                                                 start=(di == 0), stop=(di == 2),
                                                 tile_position=(64 * q, 0))
                            if (q + yb + h) % 2 == 0:
                                nc.scalar.copy(
                                    out=h1r[:, q, oy0:oy0 + 8, :], in_=pc1)
                            else:
                                nc.vector.tensor_copy(
                                    out=h1r[:, q, oy0:oy0 + 8, :], in_=pc1)

                # deferred big weight loads (overlap the BN1 collective)
                nc.sync.dma_start(out=wh_sb, in_=wh)
                nc.sync.dma_start(out=wb_sb, in_=wb)
                nc.sync.dma_start(out=wk_sb, in_=wk)

                # BN1 partial stats -> global sums via AllGather
                st1 = small.tile([128, 16, 6], F32)
                for i in range(16):
                    nc.vector.bn_stats(out=st1[:, i, :],
                                       in_=h1[:, i * 512:(i + 1) * 512])
                mv1 = small.tile([128, 2], F32)
                nc.vector.bn_aggr(out=mv1, in_=st1)
                sums1 = small.tile([128, 2], F32)
                tmp1 = small.tile([128, 1], F32)
                nc.vector.tensor_scalar_mul(out=sums1[:, 0:1], in0=mv1[:, 0:1],
                                            scalar1=8192.0)
                nc.vector.tensor_mul(out=tmp1, in0=mv1[:, 0:1], in1=mv1[:, 0:1])
                nc.vector.tensor_add(out=tmp1, in0=tmp1, in1=mv1[:, 1:2])
                nc.vector.tensor_scalar_mul(out=sums1[:, 1:2], in0=tmp1,
                                            scalar1=8192.0)
                red1 = _allgather_sum(nc, tc, dram, small, sums1, "bn1",
                                      fold_halves=True)
                s1t1 = small.tile([128, 2], F32)
                _bn_scale_bias(nc, s1t1, red1, wa32, BN1_OFF, 131072.0,
                               eps_t, small, rows=128)

                # apply BN1 + relu -> h1n (f32r), x padded to 66
                h1n = ds1.tile([128, 2 * 64 * 66], FP16)
                h1nr3 = h1n.rearrange("p (sh y x) -> p sh y x",
                                      sh=2, y=64, x=66)
                for sh in range(2):
                    for xc in (0, 65):
                        h1n_pads = bass.AP(tensor=h1n.tensor,
                                           offset=h1n.offset + 4224 * sh + xc,
                                           ap=[h1n.ap[0], [66, 64]])
                        nc.vector.memset(h1n_pads, 0.0)
                h1r4 = h1.rearrange("p (sh y x) -> p sh y x", sh=2, y=64, x=64)
                for sh in range(2):
                    for y0, y1 in ((0, 34), (34, 64)):
                        nc.scalar.activation(out=h1nr3[:, sh, y0:y1, 1:65],
                                             in_=h1r4[:, sh, y0:y1, :],
                                             func=AF.Relu,
                                             scale=s1t1[:, 0:1], bias=s1t1[:, 1:2])

                # conv2: depthwise 3x3 stride 2 -> d2 [128, (sh, 32, 32)]
                h1nr = h1n.rearrange(
                    "p (sh y2 ty x2 tx) -> p sh y2 ty x2 tx",
                    sh=2, y2=32, ty=2, x2=33, tx=2)
                d2 = ds1.tile([128, 2048], FP16)
                for sh in range(2):
                    pd2 = psB.tile([128, 1024], F32, tag="b", name="pd2")
                    pd2r = pd2.rearrange("p (h y x) -> p h y x", h=2, y=16, x=32)
                    for h in range(2):
                        for ti, (dy, dx) in enumerate(TAPS):
                            oy0, oy1 = _clip(h * 16, h * 16 + 16,
                                             1 if dy == 0 else 0, 32)
                            if dy == 1:
                                ys, par = oy0, 0
                            elif dy == 0:
                                ys, par = oy0 - 1, 1
                            else:
                                ys, par = oy0, 1
                            rhs = h1nr[:, sh, ys:ys + (oy1 - oy0), par,
                                       dx // 2:dx // 2 + 32, dx % 2]
                            outp = pd2r[:, h, oy0 - h * 16:oy1 - h * 16, :]
                            t = TAPS.index((dy, dx))
                            nc.tensor.matmul(
                                outp, wh_sb[:, W2D_OFF + t * 128:
                                            W2D_OFF + t * 128 + 128], rhs,
                                start=(ti == 0), stop=(ti == len(TAPS) - 1))
                    nc.scalar.copy(out=d2[:, sh * 1024:(sh + 1) * 1024], in_=pd2)

                # conv3: 1x1, 64 -> 128 ; h3 [128=outc, (s, 1024px)]
                for a in range(2):
                    for nb in range(4):
                        pc3 = psA.tile([128, 512], F32, tag="a",
                                       name=f"pc3_{a}_{nb}")
                        nc.tensor.matmul(
                            pc3,
                            wh_sb[64 * a:64 * a + 64, W3_OFF:W3_OFF + 128],
                            d2[64 * a:64 * a + 64, nb * 512:(nb + 1) * 512],
                            start=True, stop=True)
                        s_full = 2 * a + nb // 2
                        dst = h3[:, s_full * 1024 + (nb % 2) * 512:
                                 s_full * 1024 + (nb % 2) * 512 + 512]
                        if nb % 2 == 0:
                            nc.scalar.copy(out=dst, in_=pc3)
                        else:
                            nc.vector.tensor_copy(out=dst, in_=pc3)

                # BN2 stats + allgather
                st2 = small.tile([128, 8, 6], F32)
                for i in range(8):
                    nc.vector.bn_stats(out=st2[:, i, :],
                                       in_=h3[:, i * 512:(i + 1) * 512])
                mv2 = small.tile([128, 2], F32)
                nc.vector.bn_aggr(out=mv2, in_=st2)
                sums2 = small.tile([128, 2], F32)
                tmp2 = small.tile([128, 1], F32)
                nc.vector.tensor_scalar_mul(out=sums2[:, 0:1], in0=mv2[:, 0:1],
                                            scalar1=4096.0)
                nc.vector.tensor_mul(out=tmp2, in0=mv2[:, 0:1], in1=mv2[:, 0:1])
                nc.vector.tensor_add(out=tmp2, in0=tmp2, in1=mv2[:, 1:2])
                nc.vector.tensor_scalar_mul(out=sums2[:, 1:2], in0=tmp2,
                                            scalar1=4096.0)
                red2 = _allgather_sum(nc, tc, dram, small, sums2, "bn2")
                s2t2 = small.tile([128, 2], F32)
                _bn_scale_bias(nc, s2t2, red2, wb32, BN2_OFF, 32768.0,
                               eps_t, small, rows=128)

            # ---------------- main loop ----------------
            with (
                tc.tile_pool(name="xp", bufs=XP_BUFS) as xp,
                tc.tile_pool(name="sa", bufs=6) as sa_pool,
                tc.tile_pool(name="sb", bufs=4) as sb_pool,
                tc.tile_pool(name="sc", bufs=4) as sc_pool,
                tc.tile_pool(name="dp", bufs=8) as dp,
                tc.tile_pool(name="stp", bufs=8) as stp,
            ):
                def new_x_tile(name):
                    # pads (rows/cols 0 and 33) of every xp slot were zeroed
                    # once below; applies only write the interior.
                    return xp.tile([128, 34 * 34], FP16, tag="X", name=name)

                _dummies = []
                for i in range(XP_BUFS):
                    zt = xp.tile([128, 34 * 34], FP16, tag="X", name=f"xz{i}")
                    rows = bass.AP(tensor=zt.tensor, offset=zt.offset,
                                   ap=[zt.ap[0], [33 * 34, 2], [1, 34]])
                    cols = bass.AP(tensor=zt.tensor, offset=zt.offset,
                                   ap=[zt.ap[0], [34, 34], [33, 2]])
                    nc.vector.memset(rows, 0.0)
                    nc.vector.memset(cols, 0.0)
                    _dummies.append(zt)
                _pad_scratch = small.tile([128, 1], FP16)
                for zt in _dummies:
                    nc.vector.tensor_copy(out=_pad_scratch, in_=zt[:, 0:1])

                junk1 = small.tile([128, 1024], FP16)
                junk2 = small.tile([128, 1024], FP16)

                h3r = h3.rearrange("p (s y x) -> p s y x", s=4, y=32, x=32)
                Xcur = {}
                for s in range(BPC):
                    xt = new_x_tile(f"X1_0_{s}")
                    xtr = xt.rearrange("p (y x) -> p y x", y=34, x=34)
                    nc.scalar.activation(out=xtr[:, 1:33, 1:33],
                                         in_=h3r[:, s, :, :],
                                         func=AF.Relu,
                                         scale=s2t2[:, 0:1], bias=s2t2[:, 1:2])
                    Xcur[(0, s)] = xt

                pooled_in = small.tile([128, 2, 4, 4], F32)

                for L in range(1, LAYERS + 1):
                    gs_in = sorted({g for (g, _s) in Xcur})
                    mgs = [2, 3] if L == LAYERS else [0, 1, 2, 3]
                    Xnext = {}
                    Dall = {}

                    def emit_dw(s, L=L, gs_in=gs_in, Dall=Dall, Xcur=Xcur):
                        for g in gs_in:
                            Xr = Xcur[(g, s)].rearrange("p (y x) -> p y x",
                                                        y=34, x=34)
                            if (g in PE_G) if L > 1 else True:
                                dD = dp.tile([128, 1024], FP16, tag="D",
                                             name=f"D{L}_{s}_{g}")
                                for h in range(2):
                                    pdw = psA.tile([128, 512], F32, tag="a",
                                                   name=f"pdw{L}_{s}_{g}_{h}")
                                    pdwr = pdw.rearrange("p (y x) -> p y x",
                                                         y=16, x=32)
                                    for ti, (dy, dx) in enumerate(TAPS):
                                        rhs = Xr[:, h * 16 + dy:h * 16 + 16 + dy,
                                                 dx:dx + 32]
                                        nc.tensor.matmul(
                                            pdwr,
                                            wh_sb[:, WDW_OFF + (g * 9 + ti) * 128:
                                                  WDW_OFF + (g * 9 + ti) * 128 + 128],
                                            rhs,
                                            start=(ti == 0),
                                            stop=(ti == len(TAPS) - 1))
                                    nc.scalar.copy(
                                        out=dD[:, h * 512:h * 512 + 512],
                                        in_=pdw)
                                Dall[(g, s)] = dD
                            else:
                                # DVE products into plane stacks
                                sA = sa_pool.tile([128, 4, 1024], FP16, tag="A",
                                                  name=f"A{L}_{s}_{g}")
                                sB = sb_pool.tile([128, 4, 1024], FP16, tag="B",
                                                  name=f"B{L}_{s}_{g}")
                                sC = sc_pool.tile([128, 1024], FP16, tag="C",
                                                  name=f"C{L}_{s}_{g}")
                                for ti, (dy, dx) in enumerate(TAPS):
                                    if ti < 4:
                                        dst = sA[:, ti, :]
                                    elif ti < 8:
                                        dst = sB[:, ti - 4, :]
                                    else:
                                        dst = sC
                                    nc.vector.tensor_scalar_mul(
                                        out=dst.rearrange("p (y x) -> p y x",
                                                          y=32, x=32),
                                        in0=Xr[:, dy:dy + 32, dx:dx + 32],
                                        scalar1=wk_sb[:, g * 9 + ti:
                                                      g * 9 + ti + 1])
                                # 2 accumulate-DMAs: A[0:2] += B[0:2]; A[2:4] += B[2:4]
                                nc.gpsimd.dma_start(out=sA[:, 0:2, :],
                                                    in_=sB[:, 0:2, :],
                                                    accum_op=ALU.add)
                                nc.gpsimd.dma_start(out=sA[:, 2:4, :],
                                                    in_=sB[:, 2:4, :],
                                                    accum_op=ALU.add)
                                # 3 DVE adds: A[0:2] += A[2:4]; A0 += A1; A0 += C
                                nc.vector.tensor_add(out=sA[:, 0:2, :],
                                                     in0=sA[:, 0:2, :],
                                                     in1=sA[:, 2:4, :])
                                nc.vector.tensor_add(out=sA[:, 0, :],
                                                     in0=sA[:, 0, :],
                                                     in1=sA[:, 1, :])
                                eng = (nc.vector if L in (1, LAYERS)
                                       else nc.gpsimd)
                                eng.tensor_add(out=sA[:, 0, :],
                                               in0=sA[:, 0, :],
                                               in1=sC)
                                Dall[(g, s)] = sA

                    def emit_mix(s, L=L, gs_in=gs_in, mgs=mgs, Dall=Dall,
                                 Xnext=Xnext):
                        for mg in mgs:
                            pm = psB.tile([128, 1024], F32, tag="b",
                                          name=f"pm{L}_{s}_{mg}")
                            for h in range(2):
                                for ki, kg in enumerate(gs_in):
                                    dD = Dall[(kg, s)]
                                    rhs = (dD[:, 0, h * 512:h * 512 + 512]
                                           if len(dD.shape) == 3
                                           else dD[:, h * 512:h * 512 + 512])
                                    nc.tensor.matmul(
                                        pm[:, h * 512:h * 512 + 512],
                                        wh_sb[:, WMIX_OFF + kg * 512 + mg * 128:
                                              WMIX_OFF + kg * 512 + mg * 128 + 128],
                                        rhs,
                                        start=(ki == 0),
                                        stop=(ki == len(gs_in) - 1))
                            sc = stp.tile([128, 1], F32, tag="sc")
                            tt = stp.tile([128, 1], F32, tag="tt")
                            if mg in (0, 1) or s == 0 or L == LAYERS:
                                st = stp.tile([128, 2, 6], F32, tag="st")
                                nc.vector.bn_stats(out=st[:, 0, :],
                                                   in_=pm[:, 0:512])
                                nc.vector.bn_stats(out=st[:, 1, :],
                                                   in_=pm[:, 512:1024])
                                mv = stp.tile([128, 2], F32, tag="mv")
                                nc.vector.bn_aggr(out=mv, in_=st)
                                # sc = gamma * rsqrt(var+eps); tt = beta - mean*sc
                                nc.scalar.activation(out=sc, in_=mv[:, 1:2],
                                                     func=AF.Sqrt, bias=eps_t)
                                nc.vector.reciprocal(out=sc, in_=sc)
                                nc.vector.tensor_scalar_mul(
                                    out=sc, in0=sc,
                                    scalar1=wb32[:, NGB_OFF + 3 * mg:
                                                 NGB_OFF + 3 * mg + 1])
                                nc.vector.tensor_mul(out=tt, in0=mv[:, 0:1],
                                                     in1=sc)
                                nc.vector.tensor_scalar(
                                    out=tt, in0=tt, scalar1=-1.0,
                                    scalar2=wb32[:, NGB_OFF + 3 * mg + 1:
                                                 NGB_OFF + 3 * mg + 2],
                                    op0=ALU.mult, op1=ALU.add)
                            else:
                                # Act-engine stats: S1 = sum(pm), S2 = sum(pm^2)
                                s1 = stp.tile([128, 1], F32, tag="s1")
                                s2 = stp.tile([128, 1], F32, tag="s2")
                                nc.scalar.activation(out=junk1, in_=pm,
                                                     func=AF.Square,
                                                     accum_out=s2)
                                nc.scalar.activation(out=junk2, in_=pm,
                                                     func=AF.Identity,
                                                     accum_out=s1)
                                # V = N*S2 - S1^2 ; sc = gamma*N/sqrt(V+N^2*eps)
                                v = stp.tile([128, 1], F32, tag="v")
                                nc.vector.tensor_mul(out=v, in0=s1, in1=s1)
                                nc.vector.tensor_scalar(
                                    out=v, in0=s2, scalar1=NPX, scalar2=v,
                                    op0=ALU.mult, op1=ALU.subtract)
                                nc.scalar.activation(out=sc, in_=v,
                                                     func=AF.Sqrt, bias=epsN_t)
                                nc.vector.reciprocal(out=sc, in_=sc)
                                nc.vector.tensor_scalar_mul(
                                    out=sc, in0=sc,
                                    scalar1=wb32[:, NGB_OFF + 3 * mg + 2:
                                                 NGB_OFF + 3 * mg + 3])
                                # tt = beta - (S1/N)*sc
                                nc.vector.tensor_mul(out=tt, in0=s1, in1=sc)
                                nc.vector.tensor_scalar(
                                    out=tt, in0=tt, scalar1=-1.0 / NPX,
                                    scalar2=wb32[:, NGB_OFF + 3 * mg + 1:
                                                 NGB_OFF + 3 * mg + 2],
                                    op0=ALU.mult, op1=ALU.add)
                            if L < LAYERS:
                                xt = new_x_tile(f"X{L + 1}_{mg}_{s}")
                                Xnext[(mg, s)] = xt
                                xtr = xt.rearrange("p (y x) -> p y x",
                                                   y=34, x=34)
                                pmr2 = pm.rearrange("p (y x) -> p y x",
                                                    y=32, x=32)
                                nc.scalar.activation(
                                    out=xtr[:, 1:33, 1:33], in_=pmr2,
                                    func=AF.Relu, scale=sc, bias=tt)
                            else:
                                pmr = pm.rearrange("p (h y x) -> p h y x",
                                                   h=2, y=16, x=32)
                                nc.scalar.activation(
                                    out=pooled_in[:, mg - 2, s, :],
                                    in_=pmr[:, 0, HALF - 1:HALF + 1,
                                            HALF - 1:HALF + 1],
                                    func=AF.Identity, scale=sc, bias=tt)

                    # software-pipelined emission: dw(0) dw(1) mix(0) dw(2)
                    # mix(1) dw(3) mix(2) mix(3).  Last layer: all dw first
                    # (mix is half-sized and cannot hide the DVE chains).
                    if L == LAYERS:
                        emit_dw(0)
                        emit_dw(1)
                        emit_dw(2)
                        emit_mix(0)
                        emit_dw(3)
                        emit_mix(1)
                        emit_mix(2)
                        emit_mix(3)
                    else:
                        emit_dw(0)
                        emit_dw(1)
                        emit_mix(0)
                        emit_dw(2)
                        emit_mix(1)
                        emit_dw(3)
                        emit_mix(2)
                        emit_mix(3)
                    Xcur = Xnext

                # ---------------- readout ----------------
                tadd = small.tile([128, 2, 4], F32)
                tadd2 = small.tile([128, 2, 4], F32)
                pooled = small.tile([128, 2, 4], F32R)
                nc.vector.tensor_add(out=tadd, in0=pooled_in[:, :, :, 0],
                                     in1=pooled_in[:, :, :, 1])
                nc.vector.tensor_add(out=tadd2, in0=pooled_in[:, :, :, 2],
                                     in1=pooled_in[:, :, :, 3])
                nc.vector.tensor_add(out=pooled, in0=tadd, in1=tadd2)
                y_sb = small.tile([128, 8, 4], F32)
                for mo in range(8):
                    mlen = 128 if mo < 7 else OUT - 7 * 128
                    pf = psA.tile([128, 512], F32, tag="a", name=f"pf{mo}")
                    for kgi in range(2):
                        nc.tensor.matmul(
                            pf[0:mlen, 0:4],
                            wb_sb[:, FCW_OFF + kgi * 1000 + mo * 128:
                                  FCW_OFF + kgi * 1000 + mo * 128 + mlen],
                            pooled[:, kgi, :],
                            start=(kgi == 0), stop=(kgi == 1))
                    nc.scalar.activation(
                        out=y_sb[0:mlen, mo, :], in_=pf[0:mlen, 0:4],
                        func=AF.Identity,
                        bias=wb32[0:mlen, FCB_OFF + mo:FCB_OFF + mo + 1],
                        scale=1.0)
                    dst = bass.AP(tensor=y4.tensor, offset=128 * mo,
                                  ap=[[1, mlen], [OUT, BPC]])
                    nc.sync.dma_start(out=dst, in_=y_sb[0:mlen, mo, :])

    nc.finalize()
    return nc


def _allgather_sum(nc, tc, dram, small, sums, name, fold_halves=False):
    """AllGather the [128,2] partial sums across 8 cores and sum locally.
    With fold_halves, partition halves (64a+ch) are also summed and the
    result is duplicated to all 128 partitions.  Returns [128,2] SBUF."""
    cc_in = dram.tile([128, 2], F32, name=f"{name}_in")
    cc_out = dram.tile([N_CORES, 256], F32, name=f"{name}_out")
    nc.sync.dma_start(out=cc_in, in_=sums)
    nc.gpsimd.collective_compute(
        "AllGather", ALU.bypass,
        replica_groups=[list(range(N_CORES))],
        ins=[cc_in.opt()], outs=[cc_out.opt()])
    if fold_halves:
        # partition p reads (core r, half h) slot value [(p%64)*2 + j]
        red = small.tile([128, 16, 2], F32, name=f"{name}_red")
        for half in range(2):
            nc.sync.dma_start(
                out=red[64 * half:64 * half + 64, :, :],
                in_=bass.AP(tensor=cc_out.tensor, offset=cc_out.offset,
                            ap=[[2, 64], [128, 16], [1, 2]]))
        nc.vector.tensor_add(out=red[:, 0:8, :], in0=red[:, 0:8, :],
                             in1=red[:, 8:16, :])
    else:
        red = small.tile([128, 16, 2], F32, name=f"{name}_red")
        nc.sync.dma_start(
            out=red[:, 0:8, :],
            in_=bass.AP(tensor=cc_out.tensor, offset=cc_out.offset,
                        ap=[[2, 128], [256, N_CORES], [1, 2]]))
    nc.vector.tensor_add(out=red[:, 0:4, :], in0=red[:, 0:4, :],
                         in1=red[:, 4:8, :])
    nc.vector.tensor_add(out=red[:, 0:2, :], in0=red[:, 0:2, :],
                         in1=red[:, 2:4, :])
    nc.vector.tensor_add(out=red[:, 0, :], in0=red[:, 0, :],
                         in1=red[:, 1, :])
    return red[:, 0, :]


def di_col(dx):
    return {1: 0, 0: 1, 2: 2}[dx]


def _clip(lo, hi, lo2, hi2):
    return max(lo, lo2), min(hi, hi2)


def _bn_scale_bias(nc, out_st, sums, w32, gb_off, n_tot, eps_t, pool, rows):
    """out_st[:rows, 0] = gamma*rsqrt(var+eps); out_st[:rows, 1] = beta - mu*scale."""
    r = slice(0, rows)
    mu = pool.tile([128, 1], F32, name=f"mu{gb_off}")
    ex2 = pool.tile([128, 1], F32, name=f"ex2{gb_off}")
    var = pool.tile([128, 1], F32, name=f"var{gb_off}")
    nc.vector.tensor_scalar_mul(out=mu[r], in0=sums[r, 0:1], scalar1=1.0 / n_tot)
    nc.vector.tensor_scalar_mul(out=ex2[r], in0=sums[r, 1:2], scalar1=1.0 / n_tot)
    nc.vector.tensor_mul(out=var[r], in0=mu[r], in1=mu[r])
    nc.vector.tensor_sub(out=var[r], in0=ex2[r], in1=var[r])
    nc.scalar.activation(out=var[r], in_=var[r], func=AF.Sqrt, bias=eps_t[r])
    nc.vector.reciprocal(out=var[r], in_=var[r])
    nc.vector.tensor_scalar_mul(out=out_st[r, 0:1], in0=var[r],
                                scalar1=w32[r, gb_off:gb_off + 1])
    nc.vector.tensor_mul(out=mu[r], in0=mu[r], in1=out_st[r, 0:1])
    nc.vector.tensor_scalar(out=out_st[r, 1:2], in0=mu[r], scalar1=-1.0,
                            scalar2=w32[r, gb_off + 1:gb_off + 2],
                            op0=ALU.mult, op1=ALU.add)


def _pack_weights(ds_w1, ds_w2, ds_w3, conv_w, graph_w, fc_w, fc_b,
                  bn1_g, bn1_b, bn2_g, bn2_b, norm_g, norm_b):
    wc1 = np.zeros((128, WC1), np.float16)
    wa = np.zeros((128, WA), np.float32)
    wb = np.zeros((128, WB), np.float32)
    wh = np.zeros((128, WH), np.float16)
    wk = np.zeros((128, 36), np.float32)
    # pruned graph weight
    k = int((1.0 - PRUNE) * DIM * DIM)
    a = np.abs(graph_w).ravel()
    thresh = np.partition(a, -k)[-k]
    w_eff = np.where(np.abs(graph_w) >= thresh, graph_w, 0.0).astype(np.float32)
    # conv1 taps, paired block-diag
    for dx in range(3):
        dc = di_col(dx)
        blk = np.zeros((128, 128), np.float32)
        for qq in range(2):
            for aa in range(2):
                for dy in range(3):
                    for c in range(3):
                        blk[64 * qq + 27 * aa + 3 * dy + c,
                            64 * aa:64 * aa + 64] = ds_w1[:, c, dy, dx]
        wc1[:, W1X_OFF + dc * 128:W1X_OFF + (dc + 1) * 128] = blk.astype(np.float16)
    wa[0:64, BN1_OFF] = bn1_g
    wa[64:128, BN1_OFF] = bn1_g
    wa[0:64, BN1_OFF + 1] = bn1_b
    wa[64:128, BN1_OFF + 1] = bn1_b
    # conv2 diag-dup taps (fp16)
    for t, (dy, dx) in enumerate(TAPS):
        blk = np.zeros((128, 128), np.float16)
        d = ds_w2[:, 0, dy, dx].astype(np.float16)
        for aa in range(2):
            idx = np.arange(64)
            blk[64 * aa + idx, 64 * aa + idx] = d
        wh[:, W2D_OFF + t * 128:W2D_OFF + (t + 1) * 128] = blk
    # conv3 (fp16)
    w3 = ds_w3[:, :, 0, 0].astype(np.float16)
    wh[0:64, W3_OFF:W3_OFF + 128] = w3.T
    wh[64:128, W3_OFF:W3_OFF + 128] = w3.T
    # main dw diag taps (fp16) + tap scalar columns
    for g in range(4):
        for t, (dy, dx) in enumerate(TAPS):
            blk = np.zeros((128, 128), np.float16)
            idx = np.arange(128)
            kv = conv_w[g * 128:(g + 1) * 128, 0, dy, dx]
            blk[idx, idx] = kv.astype(np.float16)
            off = WDW_OFF + (g * 9 + t) * 128
            wh[:, off:off + 128] = blk
            wk[:, g * 9 + t] = kv
    # mix (fp16)
    weT = w_eff.T
    for kg in range(4):
        wh[:, WMIX_OFF + kg * 512:WMIX_OFF + (kg + 1) * 512] = \
            weT[kg * 128:(kg + 1) * 128, :].astype(np.float16)
    # fc (x0.25 for the 2x2 mean)
    for kg in range(2):
        wb[:, FCW_OFF + kg * 1000:FCW_OFF + (kg + 1) * 1000] = \
            0.25 * fc_w[:, kg * 128:(kg + 1) * 128].T
    fcb = np.zeros((128, 8), np.float32)
    fb = np.zeros(1024, np.float32)
    fb[:OUT] = fc_b
    fcb[:, :] = fb.reshape(8, 128).T
    wb[:, FCB_OFF:FCB_OFF + 8] = fcb
    wb[:, BN2_OFF] = bn2_g
    wb[:, BN2_OFF + 1] = bn2_b
    for g in range(4):
        wb[:, NGB_OFF + 3 * g] = norm_g[g * 128:(g + 1) * 128]
        wb[:, NGB_OFF + 3 * g + 1] = norm_b[g * 128:(g + 1) * 128]
        wb[:, NGB_OFF + 3 * g + 2] = norm_g[g * 128:(g + 1) * 128] * NPX
    return wc1, wa, wb, wh, wk


_nc_cache = None
last_results = None


def kernel(**inputs):
    global _nc_cache, last_results
    inputs = {k: np.asarray(v, np.float32) for k, v in inputs.items()}
    wc1, wa, wb, wh, wk = _pack_weights(
        inputs["ds_w1"], inputs["ds_w2"], inputs["ds_w3"], inputs["conv_w"],
        inputs["graph_w"], inputs["fc_w"], inputs["fc_b"],
        inputs["bn1_g"], inputs["bn1_b"], inputs["bn2_g"], inputs["bn2_b"],
        inputs["norm_g"], inputs["norm_b"])
    x = inputs["x"]
    if _nc_cache is None:
        _nc_cache = build_nc()
    nc = _nc_cache
    x16 = x.astype(np.float16)
    in_maps = [{"x4": np.ascontiguousarray(x16[c * BPC:(c + 1) * BPC]),
                "wc1": wc1, "wa": wa, "wb": wb, "wh": wh, "wk": wk}
               for c in range(N_CORES)]
    # Retry on non-finite output: the axon device occasionally goes through
    # transient phases of returning garbage; back off and re-run.
    for _delay in (0, 0, 15, 45, 90):
        if _delay:
            time.sleep(_delay)
        res = run_bass_kernel_spmd(nc, in_maps, core_ids=list(range(N_CORES)))
        last_results = res
        out = np.concatenate([res.results[c]["y4"] for c in range(N_CORES)],
                             axis=0)
        if np.isfinite(out).all():
            break
    return out
